# revision 19
# baseline (speedup 1.0000x reference)
"""Slot-attention module kernel (nn_AttentionModule_39084202394083) for 8x TRN2.

Contract: kernel(**inputs) takes FULL unsharded inputs, returns FULL output
[S=8, B=64, D=256] float32.

Strategy: data-parallel over batch B across 8 NeuronCores (8 batch elements
per core). Per core, a single Bass/Tile program:
  - per batch element: LayerNorm(x) (bn_stats + Newton rsqrt on DVE),
    PE-transpose to get xn^T, then k^T = wk_g.T @ xn^T and v = xn^T.T @ wv_g
    (all matmul operands bf16; fp32 matmul is 4x slower on trn2).
  - 3 slot-attention iterations: dots computed directly transposed [n, s]
    (k^T chunks as the stationary operand) so the softmax over the slot axis
    is a free-dim grouped reduce; the attention renormalization over n is
    folded into the updates matmul via an appended ones-column on v.
  - GRU + MLP on slots batched over NB=4 batch elements per group.
Host side only reshapes/shards tensors and folds tiny (O(D^2)) constants
(LN gains into weight matrices, bias vectors); all O(B*N*D) compute runs on
device.
"""

import numpy as np
from contextlib import ExitStack

import ml_dtypes

import concourse.bass as bass
import concourse.tile as tile
from concourse import bacc, mybir
from concourse.masks import make_identity
from concourse.bass_utils import run_bass_kernel_spmd

F32 = mybir.dt.float32
BF16 = mybir.dt.bfloat16
U32 = mybir.dt.uint32
AF = mybir.ActivationFunctionType
ALU = mybir.AluOpType
AX = mybir.AxisListType

NCORES = 8
B, N, D, S, H = 64, 4096, 256, 8, 1024
ITERS = 3
LN_EPS = 1e-5
ATTN_EPS = 1e-8
SCALE = float(D) ** -0.5
NCH = N // 128      # 32 n-chunks of 128 tokens
DCH = D // 128      # 2 d-chunks
HCH = H // 128      # 8 h-chunks
VROW = 258          # v row: 256 cols + ones col + pad (even for alignment)
RSQRT_MAGIC = 0x5F3759DF


def _newton_rsqrt(nc, pool, var, P, G, c_one, c_magic, tag):
    """rstd = 1/sqrt(var) on DVE only (no ACT table set needed).

    var: [P, G] fp32 tile (var + eps already included). Returns [P, G] tile.
    """
    y = pool.tile([P, G], F32, tag=tag + "_y")
    t = pool.tile([P, G], F32, tag=tag + "_t")
    vi = var.bitcast(U32)
    yi = y.bitcast(U32)
    ti = t.bitcast(U32)
    one_b = c_one[:P, :].to_broadcast([P, G])
    magic_b = c_magic[:P, :].to_broadcast([P, G])
    nc.vector.tensor_tensor(ti, vi, one_b, op=ALU.logical_shift_right)
    nc.vector.tensor_tensor(yi, magic_b, ti, op=ALU.subtract)
    for _ in range(2):
        nc.vector.tensor_tensor(t, y, y, op=ALU.mult)
        nc.vector.tensor_tensor(t, t, var, op=ALU.mult)
        nc.vector.tensor_scalar(t, t, -0.5, 1.5, op0=ALU.mult, op1=ALU.add)
        nc.vector.tensor_tensor(y, y, t, op=ALU.mult)
    return y


def build_nc(BL=8, NB=4, with_beta_v=False, with_b1=False, with_b2=False,
             reps=1):
    """Build the per-core Bass program. BL = batch elems per core.

    reps > 1 repeats the whole computation (re-loading slots each rep) so
    wall-clock deltas between reps variants isolate pure HW kernel time.
    """
    nc = bacc.Bacc()
    G = BL // NB      # number of groups
    R = NB * S        # slot rows per group

    x_d = nc.dram_tensor("x", [BL, N, D], F32, kind="ExternalInput")
    sl_d = nc.dram_tensor("slots_in", [BL * S, D], F32, kind="ExternalInput")
    wk_d = nc.dram_tensor("wk", [128, DCH, D], BF16, kind="ExternalInput")
    wv_d = nc.dram_tensor("wv", [128, DCH, D], BF16, kind="ExternalInput")
    wq_d = nc.dram_tensor("wq", [128, DCH, D], BF16, kind="ExternalInput")
    wih_d = nc.dram_tensor("wih", [128, DCH, 3 * D], BF16, kind="ExternalInput")
    whh_d = nc.dram_tensor("whh", [128, DCH, 3 * D], BF16, kind="ExternalInput")
    w1_d = nc.dram_tensor("w1", [128, DCH, H], BF16, kind="ExternalInput")
    w2_d = nc.dram_tensor("w2", [128, HCH, D], BF16, kind="ExternalInput")
    bk_d = nc.dram_tensor("beta_k", [128, DCH], F32, kind="ExternalInput")
    bq_d = nc.dram_tensor("bq_eff", [128, DCH], F32, kind="ExternalInput")
    brz_d = nc.dram_tensor("b_rz", [R, 2 * D], F32, kind="ExternalInput")
    bxn_d = nc.dram_tensor("b_xn", [R, D], F32, kind="ExternalInput")
    bhn_d = nc.dram_tensor("b_hn", [R, D], F32, kind="ExternalInput")
    bv_d = b1_d = b2_d = None
    if with_beta_v:
        bv_d = nc.dram_tensor("beta_v_bc", [128, D], F32, kind="ExternalInput")
    if with_b1:
        b1_d = nc.dram_tensor("b1_bc", [R, H], F32, kind="ExternalInput")
    if with_b2:
        b2_d = nc.dram_tensor("b2_bc", [R, D], F32, kind="ExternalInput")
    out_d = nc.dram_tensor("slots_out", [BL * S, D], F32, kind="ExternalOutput")

    with tile.TileContext(nc) as tc, ExitStack() as ctx:
        consts = ctx.enter_context(tc.tile_pool(name="consts", bufs=1))
        kvpool = ctx.enter_context(tc.tile_pool(name="kv", bufs=1))
        xslice = ctx.enter_context(tc.tile_pool(name="xslice", bufs=2))
        xstage = ctx.enter_context(tc.tile_pool(name="xstage", bufs=3))
        xnstage = ctx.enter_context(tc.tile_pool(name="xnstage", bufs=2))
        statp = ctx.enter_context(tc.tile_pool(name="stats", bufs=2))
        small = ctx.enter_context(tc.tile_pool(name="small", bufs=1))
        sweep = ctx.enter_context(tc.tile_pool(name="sweep", bufs=2))
        ps2 = ctx.enter_context(tc.tile_pool(name="ps2", bufs=2, space="PSUM"))
        ps1 = ctx.enter_context(tc.tile_pool(name="ps1", bufs=1, space="PSUM"))

        # ---------- persistent constants ----------
        ident = consts.tile([128, 128], BF16)
        make_identity(nc, ident)
        c_one = consts.tile([128, 1], U32)
        nc.vector.memset(c_one, 1)
        c_magic = consts.tile([128, 1], U32)
        nc.vector.memset(c_magic, RSQRT_MAGIC)

        wk = consts.tile([128, DCH, D], BF16)
        nc.sync.dma_start(wk, wk_d[:, :, :])
        wv = consts.tile([128, DCH, D], BF16)
        nc.sync.dma_start(wv, wv_d[:, :, :])
        wq = consts.tile([128, DCH, D], BF16)
        nc.sync.dma_start(wq, wq_d[:, :, :])
        wih = consts.tile([128, DCH, 3 * D], BF16)
        nc.sync.dma_start(wih, wih_d[:, :, :])
        whh = consts.tile([128, DCH, 3 * D], BF16)
        nc.sync.dma_start(whh, whh_d[:, :, :])
        w1 = consts.tile([128, DCH, H], BF16)
        nc.sync.dma_start(w1, w1_d[:, :, :])
        w2 = consts.tile([128, HCH, D], BF16)
        nc.sync.dma_start(w2, w2_d[:, :, :])
        beta_k = consts.tile([128, DCH], F32)
        nc.sync.dma_start(beta_k, bk_d[:, :])
        bq_eff = consts.tile([128, DCH], F32)
        nc.sync.dma_start(bq_eff, bq_d[:, :])
        b_rz = consts.tile([R, 2 * D], F32)
        nc.sync.dma_start(b_rz, brz_d[:, :])
        b_xn = consts.tile([R, D], F32)
        nc.sync.dma_start(b_xn, bxn_d[:, :])
        b_hn = consts.tile([R, D], F32)
        nc.sync.dma_start(b_hn, bhn_d[:, :])
        beta_v_bc = b1_bc = b2_bc = None
        if bv_d is not None:
            beta_v_bc = consts.tile([128, D], F32)
            nc.sync.dma_start(beta_v_bc, bv_d[:, :])
        if b1_d is not None:
            b1_bc = consts.tile([R, H], F32)
            nc.sync.dma_start(b1_bc, b1_d[:, :])
        if b2_d is not None:
            b2_bc = consts.tile([R, D], F32)
            nc.sync.dma_start(b2_bc, b2_d[:, :])

        # per-group slot tiles: keeps every slot-math tile at partition
        # base 0 (DVE/ACT lanes cannot shift partitions)
        slots_g_tiles = [
            consts.tile([R, D], F32, tag=f"slots_{g}", name=f"slots_{g}")
            for g in range(G)
        ]

        # k^T and v for the NB batch elems of the current group
        kT = kvpool.tile([128, NB, DCH, N], BF16)
        vv = kvpool.tile([128, NB, NCH, VROW], BF16)
        nc.vector.memset(vv[:, :, :, 256:257], 1.0)

        # small transpose helper: [rows, 256] bf16 -> [128, DCH, rows] bf16
        def transpose_small(src, rows, tag):
            tp = ps2.tile([128, DCH, rows], BF16, tag="mm")
            for dch in range(DCH):
                nc.tensor.transpose(
                    tp[:, dch, :], src[:, dch * 128:(dch + 1) * 128],
                    ident[0:rows, 0:rows],
                )
            dst = small.tile([128, DCH, rows], BF16, tag=tag)
            nc.vector.tensor_copy(dst, tp)
            return dst

        # slot layernorm (no gain/bias: those are folded into the consumer
        # weights host-side): [rows, 256] fp32 -> bf16 normalized
        def slot_ln(src_rows, rows, tag):
            st6 = statp.tile([rows, 6], F32, tag=tag + "_st6")
            nc.vector.bn_stats(st6, src_rows)
            mv = statp.tile([rows, 2], F32, tag=tag + "_mv")
            nc.vector.bn_aggr(mv, st6)
            var = statp.tile([rows, 1], F32, tag=tag + "_var")
            nc.vector.tensor_scalar(var, mv[:, 1:2], 1.0, LN_EPS,
                                    op0=ALU.mult, op1=ALU.add)
            rstd = _newton_rsqrt(nc, statp, var, rows, 1, c_one, c_magic,
                                 tag + "_nw")
            mz = statp.tile([rows, 1], F32, tag=tag + "_mz")
            nc.vector.tensor_tensor(mz, mv[:, 0:1], rstd, op=ALU.mult)
            out = small.tile([rows, D], BF16, tag=tag)
            nc.vector.tensor_scalar(out, src_rows, rstd, mz,
                                    op0=ALU.mult, op1=ALU.subtract)
            return out

        for rep in range(reps):
          for g in range(G):
            g0 = g * NB
            nc.sync.dma_start(slots_g_tiles[g], sl_d[g0 * S:(g0 + NB) * S, :])
            # =================== phase A: k/v for the group ===============
            for j in range(NB):
                b = g0 + j
                for st in range(8):  # 8 slices of 4 n-chunks (512 tokens)
                    xs = xstage.tile([128, 4, D], F32, tag="xs")
                    nc.gpsimd.dma_start(
                        xs,
                        x_d[b, st * 512:(st + 1) * 512, :]
                        .rearrange("(c p) d -> p c d", p=128),
                    )
                    # --- LN stats (bn_stats halves trick) ---
                    st6 = statp.tile([128, 4, 6], F32, tag="xst6")
                    for c in range(4):
                        nc.vector.bn_stats(st6[:, c, :], xs[:, c, :])
                    msum = statp.tile([128, 4], F32, tag="msum")
                    nc.vector.tensor_tensor(msum, st6[:, :, 1], st6[:, :, 4],
                                            op=ALU.add)
                    cv = statp.tile([128, 4], F32, tag="cv")
                    nc.vector.tensor_tensor(cv, st6[:, :, 2], st6[:, :, 5],
                                            op=ALU.add)
                    dm = statp.tile([128, 4], F32, tag="dm")
                    nc.vector.tensor_tensor(dm, st6[:, :, 1], st6[:, :, 4],
                                            op=ALU.subtract)
                    dm2 = statp.tile([128, 4], F32, tag="dm2")
                    nc.vector.tensor_tensor(dm2, dm, dm, op=ALU.mult)
                    var = statp.tile([128, 4], F32, tag="xvar")
                    nc.vector.tensor_scalar(var, cv, 1.0 / D, LN_EPS,
                                            op0=ALU.mult, op1=ALU.add)
                    nc.vector.scalar_tensor_tensor(var, dm2, 0.25, var,
                                                   op0=ALU.mult, op1=ALU.add)
                    rstd = _newton_rsqrt(nc, statp, var, 128, 4, c_one,
                                         c_magic, "xnw")
                    mz = statp.tile([128, 4], F32, tag="xmz")
                    nc.vector.scalar_tensor_tensor(mz, msum, 0.5, rstd,
                                                   op0=ALU.mult, op1=ALU.mult)
                    # --- apply LN -> bf16: (x*rstd - m*rstd), per-partition
                    # scale+bias in one DVE tensor_scalar op per chunk
                    xn = xnstage.tile([128, 4, D], BF16, tag="xn")
                    for c in range(4):
                        nc.vector.tensor_scalar(
                            xn[:, c, :], xs[:, c, :],
                            rstd[:, c:c + 1], mz[:, c:c + 1],
                            op0=ALU.mult, op1=ALU.subtract,
                        )
                    # --- transpose slice -> xnT [128, DCH, 512] bf16 ---
                    tp = ps2.tile([128, DCH, 4, 128], BF16, tag="tpg")
                    for c in range(4):
                        for dch in range(DCH):
                            nc.tensor.transpose(
                                tp[:, dch, c, :],
                                xn[:, c, dch * 128:(dch + 1) * 128],
                                ident,
                            )
                    xnT = xslice.tile([128, DCH, 512], BF16, tag="xnT")
                    nc.vector.tensor_copy(xnT[:, 0, :], tp[:, 0, :, :])
                    nc.scalar.copy(xnT[:, 1, :], tp[:, 1, :, :])
                    # --- k^T projection for this slice ---
                    for dpo in range(DCH):
                        pk = ps2.tile([128, 512], F32, tag="mm")
                        for dch in range(DCH):
                            nc.tensor.matmul(
                                pk,
                                lhsT=wk[:, dch, dpo * 128:(dpo + 1) * 128],
                                rhs=xnT[:, dch, :],
                                start=(dch == 0), stop=(dch == DCH - 1),
                            )
                        dst = kT[:, j, dpo, st * 512:(st + 1) * 512]
                        if dpo == 0:
                            nc.vector.tensor_scalar(
                                dst, pk, beta_k[:, dpo:dpo + 1], None,
                                op0=ALU.add)
                        else:
                            nc.scalar.activation(
                                dst, pk, AF.Identity,
                                bias=beta_k[:, dpo:dpo + 1], scale=1.0)
                    # --- v projection for this slice (natural layout) ---
                    for ci in range(4):
                        nch = st * 4 + ci
                        pv = ps2.tile([128, 256], F32, tag="mm")
                        for dch in range(DCH):
                            nc.tensor.matmul(
                                pv,
                                lhsT=xnT[:, dch, ci * 128:(ci + 1) * 128],
                                rhs=wv[:, dch, :],
                                start=(dch == 0), stop=(dch == DCH - 1),
                            )
                        dst = vv[:, j, nch, 0:256]
                        if beta_v_bc is not None:
                            nc.vector.tensor_tensor(dst, pv, beta_v_bc,
                                                    op=ALU.add)
                        elif ci % 2 == 0:
                            nc.vector.tensor_copy(dst, pv)
                        else:
                            nc.scalar.copy(dst, pv)

            # =================== iterations ===============================
            sl_g = slots_g_tiles[g]
            for it in range(ITERS):
                # ---- q for all NB batch elems ----
                sn = slot_ln(sl_g, R, "sn")
                snT = transpose_small(sn, R, "snT")
                qps = ps2.tile([128, DCH, R], F32, tag="mm")
                for dpo in range(DCH):
                    for dch in range(DCH):
                        nc.tensor.matmul(
                            qps[:, dpo, :],
                            lhsT=wq[:, dch, dpo * 128:(dpo + 1) * 128],
                            rhs=snT[:, dch, :],
                            start=(dch == 0), stop=(dch == DCH - 1),
                        )
                qT = small.tile([128, DCH, R], BF16, tag="qT")
                for dpo in range(DCH):
                    nc.vector.tensor_scalar(qT[:, dpo, :], qps[:, dpo, :],
                                            bq_eff[:, dpo:dpo + 1], None,
                                            op0=ALU.add)

                # updates live at partitions 0..7 always: [8, NB, 256]
                upd_all = small.tile([S, NB, D], BF16, tag="upd_all")
                for j in range(NB):
                    # ---- dots^T [n, s] ----
                    dps = ps2.tile([128, NCH, S], F32, tag="mm")
                    for nch in range(NCH):
                        for dch in range(DCH):
                            nc.tensor.matmul(
                                dps[:, nch, :],
                                lhsT=kT[:, j, dch, nch * 128:(nch + 1) * 128],
                                rhs=qT[:, dch, j * S:(j + 1) * S],
                                start=(dch == 0), stop=(dch == DCH - 1),
                            )
                    # ---- softmax over s (free-dim groups of 8) ----
                    e = sweep.tile([128, NCH, S], F32, tag="e")
                    nc.scalar.activation(e, dps, AF.Exp, bias=0.0, scale=SCALE)
                    den = sweep.tile([128, NCH], F32, tag="den")
                    nc.vector.tensor_reduce(den, e, axis=AX.X, op=ALU.add)
                    rden = sweep.tile([128, NCH], F32, tag="rden")
                    nc.vector.reciprocal(rden, den)
                    at32 = sweep.tile([128, NCH, S], F32, tag="at32")
                    nc.vector.tensor_tensor(at32, e,
                                            rden.to_broadcast([128, NCH, S]),
                                            op=ALU.mult)
                    attn = sweep.tile([128, NCH, S], BF16, tag="attn")
                    nc.vector.tensor_scalar(attn, at32, ATTN_EPS, None,
                                            op0=ALU.add)
                    # ---- updates (+ attn row-sum via ones column) ----
                    ups = ps2.tile([S, VROW - 1], F32, tag="mm")
                    for nch in range(NCH):
                        nc.tensor.matmul(
                            ups,
                            lhsT=attn[:, nch, :],
                            rhs=vv[:, j, nch, 0:VROW - 1],
                            start=(nch == 0), stop=(nch == NCH - 1),
                        )
                    rsum = statp.tile([S, 1], F32, tag="rsum")
                    nc.vector.reciprocal(rsum, ups[:, 256:257])
                    nc.vector.tensor_scalar(
                        upd_all[:, j, :], ups[:, 0:256],
                        rsum, None, op0=ALU.mult)

                # ---- GRU (joint over the group) ----
                # updT from per-j [8, 128] transposes (K=8)
                updT_ps = ps2.tile([128, DCH, R], BF16, tag="mm")
                for j in range(NB):
                    for dch in range(DCH):
                        nc.tensor.transpose(
                            updT_ps[:, dch, j * S:(j + 1) * S],
                            upd_all[:, j, dch * 128:(dch + 1) * 128],
                            ident[0:S, 0:S],
                        )
                updT = small.tile([128, DCH, R], BF16, tag="updT")
                nc.vector.tensor_copy(updT, updT_ps)
                sp_bf = small.tile([R, D], BF16, tag="sp_bf")
                nc.vector.tensor_copy(sp_bf, sl_g)
                spT = transpose_small(sp_bf, R, "spT")
                # r/z/n pre-activations; gx and gh accumulate into one tile
                rps = ps2.tile([R, D], F32, tag="tpg")
                zps = ps2.tile([R, D], F32, tag="tpg")
                xnps = ps2.tile([R, D], F32, tag="mm")
                hnps = ps2.tile([R, D], F32, tag="mm")
                for dch in range(DCH):
                    last = dch == DCH - 1
                    nc.tensor.matmul(rps, lhsT=updT[:, dch, :],
                                     rhs=wih[:, dch, 0:D],
                                     start=(dch == 0), stop=False)
                    nc.tensor.matmul(rps, lhsT=spT[:, dch, :],
                                     rhs=whh[:, dch, 0:D],
                                     start=False, stop=last)
                    nc.tensor.matmul(zps, lhsT=updT[:, dch, :],
                                     rhs=wih[:, dch, D:2 * D],
                                     start=(dch == 0), stop=False)
                    nc.tensor.matmul(zps, lhsT=spT[:, dch, :],
                                     rhs=whh[:, dch, D:2 * D],
                                     start=False, stop=last)
                    nc.tensor.matmul(xnps, lhsT=updT[:, dch, :],
                                     rhs=wih[:, dch, 2 * D:3 * D],
                                     start=(dch == 0), stop=last)
                    nc.tensor.matmul(hnps, lhsT=spT[:, dch, :],
                                     rhs=whh[:, dch, 2 * D:3 * D],
                                     start=(dch == 0), stop=last)
                t_rz = small.tile([R, 2 * D], F32, tag="t_rz")
                rz_sb = small.tile([R, 2 * D], F32, tag="rz_sb")
                nc.vector.tensor_tensor(rz_sb[:, 0:D], rps, b_rz[:, 0:D],
                                        op=ALU.add)
                nc.vector.tensor_tensor(rz_sb[:, D:2 * D], zps,
                                        b_rz[:, D:2 * D], op=ALU.add)
                nc.scalar.activation(t_rz, rz_sb, AF.Tanh, bias=0.0, scale=0.5)
                xn_sb = small.tile([R, D], F32, tag="xn_sb")
                nc.vector.tensor_tensor(xn_sb, xnps, b_xn, op=ALU.add)
                hn_sb = small.tile([R, D], F32, tag="hn_sb")
                nc.vector.tensor_tensor(hn_sb, hnps, b_hn, op=ALU.add)
                # r*hn = 0.5*(1+t_r)*hn ; n = tanh(xn + r*hn)
                rhn = small.tile([R, D], F32, tag="rhn")
                nc.vector.scalar_tensor_tensor(rhn, t_rz[:, 0:D], 1.0, hn_sb,
                                               op0=ALU.add, op1=ALU.mult)
                n_in = small.tile([R, D], F32, tag="n_in")
                nc.vector.scalar_tensor_tensor(n_in, rhn, 0.5, xn_sb,
                                               op0=ALU.mult, op1=ALU.add)
                t_n = small.tile([R, D], F32, tag="t_n")
                nc.scalar.activation(t_n, n_in, AF.Tanh, bias=0.0, scale=1.0)
                # slots_mid = n + z*(sp-n), z = 0.5*(1+t_z)
                d1 = small.tile([R, D], F32, tag="d1")
                nc.vector.tensor_tensor(d1, sl_g, t_n, op=ALU.subtract)
                d2 = small.tile([R, D], F32, tag="d2")
                nc.vector.tensor_tensor(d2, d1, t_rz[:, D:2 * D], op=ALU.mult)
                s12 = small.tile([R, D], F32, tag="s12")
                nc.vector.tensor_tensor(s12, d1, d2, op=ALU.add)
                smid = small.tile([R, D], F32, tag="smid")
                nc.vector.scalar_tensor_tensor(smid, s12, 0.5, t_n,
                                               op0=ALU.mult, op1=ALU.add)
                # ---- MLP ----
                ff = slot_ln(smid, R, "ff")
                ffT = transpose_small(ff, R, "ffT")
                h1ps = ps1.tile([R, H], F32, tag="mlp")
                for dch in range(DCH):
                    for half in range(2):
                        nc.tensor.matmul(
                            h1ps[:, half * 512:(half + 1) * 512],
                            lhsT=ffT[:, dch, :],
                            rhs=w1[:, dch, half * 512:(half + 1) * 512],
                            start=(dch == 0), stop=(dch == DCH - 1),
                        )
                h1r = small.tile([R, H], BF16, tag="h1r")
                if b1_bc is not None:
                    h1b = small.tile([R, H], F32, tag="h1b")
                    nc.vector.tensor_tensor(h1b, h1ps, b1_bc, op=ALU.add)
                    nc.scalar.activation(h1r, h1b, AF.Relu, bias=0.0,
                                         scale=1.0)
                else:
                    nc.scalar.activation(h1r, h1ps, AF.Relu, bias=0.0,
                                         scale=1.0)
                # transpose h1r -> [128, HCH, R]
                h1tp = ps2.tile([128, HCH, R], BF16, tag="mm")
                for hch in range(HCH):
                    nc.tensor.transpose(
                        h1tp[:, hch, :],
                        h1r[:, hch * 128:(hch + 1) * 128],
                        ident[0:R, 0:R],
                    )
                h1rT = small.tile([128, HCH, R], BF16, tag="h1rT")
                nc.vector.tensor_copy(h1rT, h1tp)
                h2ps = ps2.tile([R, D], F32, tag="mm")
                for hch in range(HCH):
                    nc.tensor.matmul(h2ps, lhsT=h1rT[:, hch, :],
                                     rhs=w2[:, hch, :],
                                     start=(hch == 0), stop=(hch == HCH - 1))
                if b2_bc is not None:
                    tmp2 = small.tile([R, D], F32, tag="tmp2")
                    nc.vector.tensor_tensor(tmp2, h2ps, b2_bc, op=ALU.add)
                    nc.vector.tensor_tensor(sl_g, smid, tmp2, op=ALU.add)
                else:
                    nc.vector.tensor_tensor(sl_g, smid, h2ps, op=ALU.add)

            # write out this group's final slots
            nc.sync.dma_start(out_d[g0 * S:(g0 + NB) * S, :], sl_g)

    nc.compile()
    return nc


# ----------------------------------------------------------------------------
# host side
# ----------------------------------------------------------------------------
_NC_CACHE = {}
LAST_RESULTS = None


def _get_nc(BL, NB, flags):
    key = (BL, NB, flags)
    if key not in _NC_CACHE:
        _NC_CACHE[key] = build_nc(BL, NB, *flags)
    return _NC_CACHE[key]


def _bf16(a):
    return np.ascontiguousarray(a.astype(ml_dtypes.bfloat16))


def _wlayout(w):
    """[D, M] weight -> [128, DCH, M] (partition = d within chunk)."""
    Din, M = w.shape
    return np.ascontiguousarray(
        w.reshape(Din // 128, 128, M).transpose(1, 0, 2))


def prepare_host_inputs(inputs, NB=4):
    """Fold tiny constants and lay out shared (per-core-identical) tensors."""
    f = {k: np.asarray(v, np.float32) for k, v in inputs.items()}
    R = NB * S
    wk_g = f["g_in"][:, None] * f["wk"]
    wv_g = f["g_in"][:, None] * f["wv"]
    wq_e = f["g_sl"][:, None] * f["wq"]
    w1_e = f["g_ff"][:, None] * f["mlp_w1"]
    beta_k = f["b_in"] @ f["wk"] + f["bk"]
    beta_v = f["b_in"] @ f["wv"] + f["bv"]
    bq_eff = f["b_sl"] @ f["wq"] + f["bq"]
    b1_eff = f["b_ff"] @ f["mlp_w1"] + f["mlp_b1"]
    b2_eff = f["mlp_b2"]
    b_sum = f["b_ih"] + f["b_hh"]

    shared = {
        "wk": _bf16(_wlayout(wk_g)),
        "wv": _bf16(_wlayout(wv_g)),
        "wq": _bf16(_wlayout(wq_e)),
        "wih": _bf16(_wlayout(f["w_ih"])),
        "whh": _bf16(_wlayout(f["w_hh"])),
        "w1": _bf16(_wlayout(w1_e)),
        "w2": _bf16(_wlayout(f["mlp_w2"])),
        "beta_k": np.ascontiguousarray(beta_k.reshape(DCH, 128).T),
        "bq_eff": np.ascontiguousarray(bq_eff.reshape(DCH, 128).T),
        "b_rz": np.ascontiguousarray(
            np.broadcast_to(b_sum[0:2 * D], (R, 2 * D))),
        "b_xn": np.ascontiguousarray(
            np.broadcast_to(f["b_ih"][2 * D:3 * D], (R, D))),
        "b_hn": np.ascontiguousarray(
            np.broadcast_to(f["b_hh"][2 * D:3 * D], (R, D))),
    }
    flags = (bool(np.any(beta_v)), bool(np.any(b1_eff)), bool(np.any(b2_eff)))
    if flags[0]:
        shared["beta_v_bc"] = np.ascontiguousarray(
            np.broadcast_to(beta_v, (128, D)))
    if flags[1]:
        shared["b1_bc"] = np.ascontiguousarray(np.broadcast_to(b1_eff, (R, H)))
    if flags[2]:
        shared["b2_bc"] = np.ascontiguousarray(np.broadcast_to(b2_eff, (R, D)))
    return shared, flags


def kernel(inputs, slots, **params):
    inputs = np.asarray(inputs, np.float32)
    slots = np.asarray(slots, np.float32)
    NB = 4
    BL = B // NCORES
    shared, flags = prepare_host_inputs(params, NB=NB)
    nc = _get_nc(BL, NB, flags)

    in_maps = []
    for c in range(NCORES):
        bsl = slice(c * BL, (c + 1) * BL)
        sl_c = np.ascontiguousarray(
            slots[:, bsl, :].transpose(1, 0, 2).reshape(BL * S, D))
        m = {"x": np.ascontiguousarray(inputs[bsl]), "slots_in": sl_c}
        m.update(shared)
        in_maps.append(m)

    res = run_bass_kernel_spmd(nc, in_maps, list(range(NCORES)))
    global LAST_RESULTS
    LAST_RESULTS = res
    out = np.empty((S, B, D), np.float32)
    for c in range(NCORES):
        o = res.results[c]["slots_out"].reshape(BL, S, D)
        out[:, c * BL:(c + 1) * BL, :] = o.transpose(1, 0, 2)
    return out
